# revision 1
# baseline (speedup 1.0000x reference)
"""Trainium2 Bass kernel for the C-LIF spiking-neuron forward pass.

Problem: x [16, 8192, 200] fp32, scalar decays dm=0.9, ds=0.6, VTH=0.5.
Per neuron, over time t:
    M = dm*(M + x_t); S = ds*(S + x_t); E = dm*E + o_prev*VTH
    u = M - S - E;    o_t = (u - VTH > 0)

On-chip reformulation (fp32, numerically faithful to the reference up to
~1-ulp reassociation at the spike threshold):
  * 2*(M_t - S_t) = 0.6 * y2_t where y2 = cascade of two one-pole IIRs on
    RAW x (the transfer function of M-S has a constant numerator).  Each
    pole is one DVE `tensor_tensor_scan` per [128, 200] tile; the gain is
    folded into the consumers, so phase B is DMA -> DVE only, with
    ScalarE/GPSIMD doing pure transpose-copies into a time-major buffer.
  * With F := E/VTH (VTH=0.5 is a power of two => exact scaling):
        F_t = dm*F_{t-1} + o_{t-1},   o_t = ((F_t + 1) < 0.6*y2_t)
    Two runtime-registered custom DVE ops:
      step:  F_t = dm*F_{t-1} + ((F_{t-1} + 1) < 0.6*Src1)   (1 op/step)
      recon: o_t = (F_t + 1) < 0.6*Src1                      (wide chunks)
    The compares are the same fused fp32 ALU expression, so the spikes
    the state update saw and the emitted spikes are bit-identical.
    F (fp32) and o (bf16; exact for 0/1) live in 4-deep column rings;
    ScalarE+GPSIMD transpose each o chunk into the neuron-major bf16
    output buffer, which leaves in ONE big DMA.

Sharding: 131072 neuron rows split evenly across 8 cores (data parallel,
no cross-device communication).  Host upcasts bf16 spikes to fp32.
"""

import numpy as np

# ---------------------------------------------------------------- constants
B, N, T = 16, 8192, 200
DM, DS, VTH = 0.9, 0.6, 0.5
GAIN = 2.0 * (DM - DS)            # 0.6: folded into the compares
N_CORES = 8
ROWS = B * N                      # 131072 neuron rows
ROWS_PER_CORE = ROWS // N_CORES   # 16384
G = ROWS_PER_CORE // 128          # 128 groups of 128 neurons
GB = 4                            # groups per DMA batch
NB = G // GB                      # 32 batches
TCH = 10                          # time-chunk (columns per recon chunk)
NCH = T // TCH                    # 20 chunks
NHALF = 4                         # ring depth in chunks
RING = NHALF * TCH                # 40 columns

_cached = {}


def _register_ops():
    """Runtime-register the two fused LIF ops."""
    from concourse import dve_ops
    from concourse.dve_spec import Spec, Src0, Src1, C0, C2, One, lower
    from concourse.dve_uop import DveOpSpec

    def reg(name, spec):
        for op in dve_ops.OPS:
            if op.name == name:
                return op
        row = dve_ops._CUSTOM_DVE_ROW_BASE + len(dve_ops.OPS)
        dve_ops._SUB_OPCODE_FOR_NAME[name] = row
        shas = {
            ver: DveOpSpec(name=name, opcode=row, uops=lower(spec, ver=ver),
                           rd1_en=True).sha(ver)
            for ver in ("v3", "v4")
        }
        op = dve_ops.DveOp(name, spec, subdim=False, uops_sha=shas)
        dve_ops.OPS.append(op)
        return op

    step = reg("LIF_STEP2_ANT", Spec(
        body=Src0 * C0 + ((Src0 + One) < Src1 * C2),
        reference=lambda in0, in1, s0, s1, imm2: in0 * s0
        + ((in0 + np.float32(1.0)) < in1 * np.float32(imm2)).astype(np.float32),
    ))
    recon = reg("LIF_RECON_ANT", Spec(
        body=(Src0 + One) < Src1 * C2,
        reference=lambda in0, in1, s0, s1, imm2:
        ((in0 + np.float32(1.0)) < in1 * np.float32(imm2)).astype(np.float32),
    ))
    return step, recon


def _build_program(iters: int = 1, phases: str = "full", timing: bool = False):
    import concourse.mybir as mybir
    from concourse import bacc, tile
    from contextlib import nullcontext

    fp32 = mybir.dt.float32
    bf16 = mybir.dt.bfloat16
    Alu = mybir.AluOpType
    step_op, recon_op = _register_ops()

    nc = bacc.Bacc("TRN2", target_bir_lowering=False, debug=False)
    if timing:
        # tiny external I/O + internal DRAM scratch: same on-device work,
        # no host<->device transfer noise in wall-clock measurements
        nc.dram_tensor("x", [128, T], fp32, kind="ExternalInput")
        o_ext = nc.dram_tensor("o", [128, T], bf16, kind="ExternalOutput").ap()
        x_d = nc.dram_tensor("xs", [ROWS_PER_CORE, T], fp32).ap()
        o_d = nc.dram_tensor("os", [ROWS_PER_CORE, T], bf16).ap()
    else:
        x_d = nc.dram_tensor("x", [ROWS_PER_CORE, T], fp32,
                             kind="ExternalInput").ap()
        o_d = nc.dram_tensor("o", [ROWS_PER_CORE, T], bf16,
                             kind="ExternalOutput").ap()

    do_b = "B" in phases or phases == "full"
    do_c = "C" in phases or phases == "full"
    do_d = "D" in phases or phases == "full"

    with tile.TileContext(nc) as tc:
        with (
            tc.tile_pool(name="xb", bufs=3) as xb_pool,
            tc.tile_pool(name="ys", bufs=3) as y_pool,
            tc.tile_pool(name="big", bufs=1) as big_pool,
            tc.tile_pool(name="consts", bufs=1) as const_pool,
        ):
            abig = big_pool.tile([128, T * G], fp32)       # [p,(t g)] y2
            onat = big_pool.tile([128, G * T], bf16)       # [p,(g t)] spikes
            # rings as SEPARATE tiles per chunk-slot so the scheduler's
            # dependency tracking is per-chunk, not whole-ring
            fr4 = [big_pool.tile([128, TCH * G], fp32, name=f"fr{i}",
                                 tag=f"fr{i}") for i in range(NHALF)]
            or4 = [big_pool.tile([128, TCH * G], bf16, name=f"or{i}",
                                 tag=f"or{i}") for i in range(NHALF)]
            dm1 = const_pool.tile([128, 1], fp32)
            ds1 = const_pool.tile([128, 1], fp32)

            a3 = abig[:].rearrange("p (t g) -> p t g", g=G)
            o3t = onat[:].rearrange("p (g t) -> p t g", t=T)

            def fcol(t):
                h, c = (t // TCH) % NHALF, t % TCH
                return fr4[h][:, c * G:(c + 1) * G]

            loop_cm = tc.For_i(0, iters, 1) if iters > 1 else nullcontext()
            with loop_cm:
                nc.vector.memset(dm1[:], DM)
                nc.vector.memset(ds1[:], DS)
                nc.vector.memset(fcol(0), 0.0)             # F_0 = 0
                if not do_b:
                    nc.vector.memset(abig[:], 0.5)
                if not do_c:
                    nc.vector.memset(onat[:], 0.0)

                # -- phase B: DMA -> DVE scans (raw x); pure transpose-copies
                #    on ScalarE/GPSIMD into the time-major y2 buffer
                for gb in range(NB if do_b else 0):
                    xb = xb_pool.tile([128, GB * T], fp32)
                    src = x_d[gb * GB * 128:(gb + 1) * GB * 128, :]
                    nc.sync.dma_start(
                        xb[:].rearrange("p (gs t) -> p gs t", t=T),
                        src.rearrange("(gs p) t -> p gs t", p=128))
                    for gs in range(GB):
                        g = gb * GB + gs
                        y1 = y_pool.tile([128, T], fp32)
                        nc.vector.tensor_tensor_scan(
                            y1[:], dm1[:].broadcast_to([128, T]),
                            xb[:, gs * T:(gs + 1) * T],
                            0.0, Alu.mult, Alu.add)
                        nc.vector.tensor_tensor_scan(
                            y1[:], ds1[:].broadcast_to([128, T]), y1[:],
                            0.0, Alu.mult, Alu.add)
                        if g % 2 == 0:
                            nc.scalar.copy(a3[:, :, g], y1[:])
                        else:
                            nc.gpsimd.tensor_copy(a3[:, :, g], y1[:])

                # -- phase C: one fused DVE op per step; chunked recon;
                #    ScalarE+GPSIMD transpose each chunk into onat
                for c in range(NCH if do_c else 0):
                    lo, hi = c * TCH, (c + 1) * TCH
                    half = (c % NHALF) * TCH
                    h = c % NHALF
                    for t in range(max(1, lo), hi):
                        nc.vector._custom_dve(
                            step_op,
                            out=fcol(t),
                            in0=fcol(t - 1),
                            in1=a3[:, t - 1, :],
                            s0=DM, imm2=GAIN)
                    nc.vector._custom_dve(
                        recon_op,
                        out=or4[h][:, :],
                        in0=fr4[h][:, :],
                        in1=abig[:, lo * G:hi * G],
                        imm2=GAIN)
                    orv = or4[h][:].rearrange("p (t g) -> p t g", g=G)
                    GS = 80    # ScalarE takes 80 groups, Pool 48
                    nc.scalar.copy(o3t[:, lo:hi, 0:GS], orv[:, :, 0:GS])
                    nc.gpsimd.tensor_copy(o3t[:, lo:hi, GS:G], orv[:, :, GS:G])

                # -- phase D: ONE big DMA for all spikes
                if do_d:
                    nc.sync.dma_start(
                        o_d.rearrange("(gs p) t -> p gs t", p=128),
                        onat[:].rearrange("p (gs t) -> p gs t", t=T))
                if timing:
                    nc.sync.dma_start(o_ext[:, :], onat[:, 0:T])

    nc.compile()
    return nc


def _run(x_flat: np.ndarray, iters: int = 1, trace: bool = False,
         phases: str = "full", timing: bool = False):
    from concourse.bass_utils import run_bass_kernel_spmd

    key = f"nc{iters}-{phases}-{timing}"
    if key not in _cached:
        _cached[key] = _build_program(iters, phases, timing)
    nc = _cached[key]
    if timing:
        in_maps = [{"x": np.zeros((128, T), np.float32)}
                   for _ in range(N_CORES)]
    else:
        shards = [
            np.ascontiguousarray(
                x_flat[c * ROWS_PER_CORE:(c + 1) * ROWS_PER_CORE])
            for c in range(N_CORES)
        ]
        in_maps = [{"x": s} for s in shards]
    res = run_bass_kernel_spmd(nc, in_maps, list(range(N_CORES)), trace=trace)
    outs = [np.asarray(r["o"], dtype=np.float32) for r in res.results]
    return np.concatenate(outs, axis=0), res


def kernel(x, decay_m=None, decay_s=None):
    x = np.asarray(x, dtype=np.float32)
    out_flat, _ = _run(x.reshape(ROWS, T))
    return out_flat.reshape(B, N, T)



# revision 4
# speedup vs baseline: 4.1174x; 4.1174x over previous
"""Trainium2 Bass kernel for the C-LIF spiking-neuron forward pass.

Problem: x [16, 8192, 200] fp32, scalar decays dm=0.9, ds=0.6, VTH=0.5.
Per neuron, over time t:
    M = dm*(M + x_t); S = ds*(S + x_t); E = dm*E + o_prev*VTH
    u = M - S - E;    o_t = (u - VTH > 0)

Reformulation (exact in real arithmetic; fp32 roundings differ from the
reference by ~1 ulp, flipping only a handful of spikes):
    2*(M-S) = 0.6*y2 where y2 = two-pole IIR cascade (dm, ds) on x.
    With v := y1 - 0.4, h := 0.6*y2 - 1, F := E/VTH, and the prescale
    xa := 0.6*x - 0.04 (constants absorbed into shifted states):
        v[t] = dm*v[t-1] + xa[t]        v[-1] = -0.4
        h[t] = ds*h[t-1] + v[t]         h[-1] = -1
        F[t] = dm*F[t-1] + o[t-1]       F[-1] = o[-1] = 0
        o[t] = (F[t] < h[t])

The whole recurrence runs as ONE hand-built custom DVE uop program at
1 element/cycle: neuron rows are processed two-at-a-time, interleaved
element-wise, so the DVE's NEXT_ALU_OUT_A/B backward feedback paths
(2-cycle latency) deliver exactly the t-1 state with zero bubbles.
Per-pair state reset rides the SUB_DIM_DONE trigger; the first two
elements of each segment run a boundary uop that substitutes the
initial state for the stale feedback flops.

Per core: DMA-in -> ScalarE/GPSIMD affine+pair-interleave -> fused DVE
scan (bf16 spikes out) -> DMA-out.  The kernel is DMA-bound (~20 MB of
HBM traffic per core).  Host side: pair-uninterleave + upcast to fp32.

Sharding: 131072 neuron rows split evenly across 8 cores (data
parallel, no cross-device communication).
"""

import numpy as np

# ---------------------------------------------------------------- constants
B, N, T = 16, 8192, 200
N_CORES = 8
ROWS = B * N                      # 131072 neuron rows
ROWS_PER_CORE = ROWS // N_CORES   # 16384
G = ROWS_PER_CORE // 128          # 128 groups of 128 neurons
NPAIR = G // 2                    # 64 interleaved pairs
SEG = 2 * T                       # 400: elements per pair segment
GB = 8                            # groups per DMA batch
NB = G // GB                      # 16 DMA-in batches
PB = 16                           # pairs per DVE op
ND = NPAIR // PB                  # 4 DVE ops / out-DMA chunks

DM = np.float32(0.9)
DS = np.float32(0.6)
GAIN = np.float32(0.6)            # 2*(dm-ds)
ABIAS = np.float32(-0.04)         # 0.4*(dm-1): affine bias for xa
V_INIT = np.float32(-0.4)
C2_VAL = np.float32(DM * V_INIT)  # dm*v_init: boundary vm value

_cached = {}

# ------------------------------------------------------------ custom DVE op
LANE_XA, LANE_DM, LANE_DS, LANE_ZERO, LANE_V, LANE_H = 0, 1, 2, 3, 4, 5


def _build_lif_uops():
    from concourse.dve_uop import (
        ENABLE,
        AluInp,
        AluOp,
        DelayInp,
        InpSel,
        OutPath,
        OutSel,
        Trigger,
        UopConfig,
        UopDpConfig,
    )

    def datapath(boundary):
        b = [UopDpConfig() for _ in range(8)]
        for st in range(8):
            b[st].pass_through_delay(LANE_XA, LANE_DM, LANE_DS, LANE_ZERO)
        b[2].enable_delay_from_src(DelayInp.PREV_ALU_OUT, LANE_V)
        for st in range(3, 8):
            b[st].pass_through_delay(LANE_V)
        b[4].enable_delay_from_src(DelayInp.PREV_ALU_OUT, LANE_H)
        for st in range(5, 8):
            b[st].pass_through_delay(LANE_H)

        if boundary:
            b[0].enable_alu(AluOp.BYPASS, AluInp.PREV_ALU_OUT)  # slot0 = C2
        else:
            b[0].enable_alu(AluOp.MULTIPLY, AluInp.PREV_ALU_OUT, AluInp.NEXT_ALU_OUT_A)
        b[1].enable_alu(AluOp.ADD, AluInp.PREV_ALU_OUT, AluInp.PREV_DELAY_0)
        b[1].alu_out_a_enable = ENABLE
        if boundary:
            b[2].enable_alu(AluOp.SUBTRACT, AluInp.PREV_DELAY_3, AluInp.PREV_DELAY_2)
        else:
            b[2].enable_alu(AluOp.MULTIPLY, AluInp.PREV_DELAY_2, AluInp.NEXT_ALU_OUT_A)
        b[3].enable_alu(AluOp.ADD, AluInp.PREV_ALU_OUT, AluInp.PREV_DELAY_4)
        b[3].alu_out_a_enable = ENABLE
        if boundary:
            b[4].enable_alu(AluOp.BYPASS, AluInp.PREV_DELAY_3)
        else:
            b[4].enable_alu(AluOp.MULTIPLY, AluInp.PREV_DELAY_1, AluInp.NEXT_ALU_OUT_A)
        if boundary:
            b[5].enable_alu(AluOp.BYPASS, AluInp.PREV_ALU_OUT)
        else:
            b[5].enable_alu(AluOp.ADD, AluInp.PREV_ALU_OUT, AluInp.NEXT_ALU_OUT_B)
        b[5].alu_out_a_enable = ENABLE
        b[6].enable_alu(AluOp.IS_LT, AluInp.PREV_ALU_OUT, AluInp.PREV_DELAY_5)
        b[6].alu_out_b_enable = ENABLE
        b[7].pass_through_alu()
        return b

    def mk(boundary):
        u = UopConfig()
        u.enable_input(InpSel.CONST_2 if boundary else InpSel.CONST_0, 0)
        u.enable_input(InpSel.SRC_0, LANE_XA + 1)
        u.enable_input(InpSel.CONST_0, LANE_DM + 1)
        u.enable_input(InpSel.CONST_1, LANE_DS + 1)
        u.enable_input(InpSel.ZERO, LANE_ZERO + 1)
        u.datapath_config = datapath(boundary)
        u.enable_output(OutSel.ALU_OUT, OutPath.WR0_LO)
        u.require_inp0 = 1
        if boundary:
            u.repeat_count = 2
            u.trigger = (Trigger.SRC_TENSOR_DONE, Trigger.SUB_DIM_DONE, Trigger.COUNT)
            u.next_uop = (0, 1, 2)
        else:
            u.trigger = (Trigger.SRC_TENSOR_DONE, Trigger.SUB_DIM_DONE, Trigger.NONE)
            u.next_uop = (0, 1, 0)
        return u

    return [mk(True), mk(True), mk(False)]


def _lif_ref_stream(xa, seg=SEG):
    """Numpy oracle of the fused op's stream semantics (CoreSim only)."""
    P, TOT = xa.shape
    x4 = xa.reshape(P, TOT // seg, seg // 2, 2)
    v = np.full(x4.shape[:2] + (2,), V_INIT, np.float32)
    h = np.full_like(v, np.float32(-1.0))
    F = np.zeros_like(v)
    o = np.zeros_like(v)
    out = np.zeros_like(x4)
    for t in range(seg // 2):
        vm = (DM * v).astype(np.float32)
        if t == 0:
            vm[...] = C2_VAL
        v = (vm + x4[:, :, t, :]).astype(np.float32)
        hm = (DS * h).astype(np.float32)
        h = (hm + v).astype(np.float32)
        Fm = (DM * F).astype(np.float32)
        F = (Fm + o).astype(np.float32)
        o = (F < h).astype(np.float32)
        out[:, :, t, :] = o
    return out.reshape(P, TOT)


def _register_lif_op():
    from concourse import dve_ops
    from concourse.dve_spec import C0, C1, C2, Spec, Src0
    from concourse.dve_uop import DveOpSpec

    name = "LIF_FUSED_SCAN_ANT"
    for op in dve_ops.OPS:
        if op.name == name:
            return op
    row = dve_ops._CUSTOM_DVE_ROW_BASE + len(dve_ops.OPS)
    dve_ops._SUB_OPCODE_FOR_NAME[name] = row
    uops = _build_lif_uops()
    spec = Spec(
        body=Src0 * C0 + C1 + C2,  # placeholder: leaf set only
        reference=lambda in0, in1, s0, s1, imm2: _lif_ref_stream(
            in0.reshape(in0.shape[0], -1)
        ),
    )
    # uops_sha deliberately invalid: compile() must never fall through to
    # lower() -- the pre-populated cache below is the only source of uops.
    op = dve_ops.DveOp(
        name, spec, subdim=True,
        uops_sha={"v3": "PINNED-BY-CACHE", "v4": "PINNED-BY-CACHE"},
    )
    for ver in ("v3", "v4"):
        s = DveOpSpec(name=name, opcode=row, uops=uops, rd1_en=False)
        s.validate(ver)
        dve_ops._COMPILE_CACHE[(name, ver)] = s
    dve_ops.OPS.append(op)
    return op


# ------------------------------------------------------------- bass program
def _build_program(iters: int = 1, timing: bool = False):
    import concourse.mybir as mybir
    from concourse import bacc, tile
    from contextlib import nullcontext

    fp32 = mybir.dt.float32
    bf16 = mybir.dt.bfloat16
    op = _register_lif_op()

    nc = bacc.Bacc("TRN2", target_bir_lowering=False, debug=False)
    if timing:
        # tiny external I/O + internal DRAM scratch: same on-device work,
        # no host<->device transfer noise in wall-clock measurements
        nc.dram_tensor("x", [128, T], fp32, kind="ExternalInput")
        o_ext = nc.dram_tensor("o", [128, T], bf16, kind="ExternalOutput").ap()
        x_d = nc.dram_tensor("xs", [ROWS_PER_CORE, T], fp32).ap()
        o_d = nc.dram_tensor("os", [ROWS_PER_CORE // 2, SEG], bf16).ap()
    else:
        x_d = nc.dram_tensor("x", [ROWS_PER_CORE, T], fp32,
                             kind="ExternalInput").ap()
        o_d = nc.dram_tensor("o", [ROWS_PER_CORE // 2, SEG], bf16,
                             kind="ExternalOutput").ap()

    o_d3 = o_d.rearrange("(s p) n -> p s n", p=128)  # [128, 64, 400]

    with tile.TileContext(nc) as tc:
        with (
            tc.tile_pool(name="xb", bufs=3) as xb_pool,
            tc.tile_pool(name="xa", bufs=3) as xa_pool,
            tc.tile_pool(name="big", bufs=1) as big_pool,
        ):
            o_il = big_pool.tile([128, G * T], bf16)   # interleaved spikes
            bias1 = big_pool.tile([128, 1], fp32)

            loop_cm = tc.For_i(0, iters, 1) if iters > 1 else nullcontext()
            with loop_cm:
                nc.vector.memset(bias1[:], float(ABIAS))
                for q in range(ND):                    # 4 chunks of 16 pairs
                    xa_t = xa_pool.tile([128, PB * SEG], fp32)
                    # interleaved view [p, s_local, j, t] for affine writes
                    xa_v = xa_t[:].rearrange(
                        "p (s t j) -> p s j t", t=T, j=2)
                    for b in range(PB * 2 // GB):      # 2 DMA batches / chunk
                        gb = q * (PB * 2 // GB) + b
                        xb = xb_pool.tile([128, GB * T], fp32)
                        src = x_d[gb * GB * 128:(gb + 1) * GB * 128, :]
                        nc.sync.dma_start(
                            xb[:].rearrange("p (gs t) -> p gs t", t=T),
                            src.rearrange("(gs p) t -> p gs t", p=128))
                        xbv = xb[:].rearrange(
                            "p (s j t) -> p s j t", j=2, t=T)
                        dst = xa_v[:, b * (GB // 2):(b + 1) * (GB // 2)]
                        # affine xa = 0.6*x - 0.04, pair-interleave layout;
                        # alternate ScalarE / GPSIMD per batch
                        if gb % 2 == 0:
                            nc.scalar.activation(
                                dst, xbv,
                                mybir.ActivationFunctionType.Identity,
                                bias=bias1[:], scale=float(GAIN))
                        else:
                            nc.gpsimd.tensor_scalar(
                                dst, xbv, float(GAIN), float(ABIAS),
                                mybir.AluOpType.mult, mybir.AluOpType.add)
                    nc.vector._custom_dve(
                        op,
                        out=o_il[:, q * PB * SEG:(q + 1) * PB * SEG],
                        in0=xa_t[:].rearrange("p (s n) -> p s n", n=SEG),
                        s0=float(DM), s1=float(DS), imm2=float(C2_VAL))
                    nc.sync.dma_start(
                        o_d3[:, q * PB:(q + 1) * PB, :],
                        o_il[:, q * PB * SEG:(q + 1) * PB * SEG].rearrange(
                            "p (s n) -> p s n", n=SEG))
                if timing:
                    nc.sync.dma_start(o_ext[:, :], o_il[:, 0:T])

    nc.compile()
    return nc


def _run(x_flat: np.ndarray, iters: int = 1, trace: bool = False,
         phases: str = "full", timing: bool = False):
    from concourse.bass_utils import run_bass_kernel_spmd

    key = f"nc{iters}-{timing}"
    if key not in _cached:
        _cached[key] = _build_program(iters, timing)
    nc = _cached[key]
    if timing:
        in_maps = [{"x": np.zeros((128, T), np.float32)}
                   for _ in range(N_CORES)]
    else:
        shards = [
            np.ascontiguousarray(
                x_flat[c * ROWS_PER_CORE:(c + 1) * ROWS_PER_CORE])
            for c in range(N_CORES)
        ]
        in_maps = [{"x": s} for s in shards]
    res = run_bass_kernel_spmd(nc, in_maps, list(range(N_CORES)), trace=trace)
    if timing:
        return None, res
    outs = []
    for r in res.results:
        od2 = np.asarray(r["o"], dtype=np.float32)  # [8192, 400] interleaved
        # rows = (s p), cols = (t j)  ->  rows (2s+j)*128+p, cols t
        o4 = od2.reshape(NPAIR, 128, T, 2)
        outs.append(np.ascontiguousarray(
            o4.transpose(0, 3, 1, 2)).reshape(ROWS_PER_CORE, T))
    return np.concatenate(outs, axis=0), res


def kernel(x, decay_m=None, decay_s=None):
    x = np.asarray(x, dtype=np.float32)
    out_flat, _ = _run(x.reshape(ROWS, T))
    return out_flat.reshape(B, N, T)
